# revision 10
# baseline (speedup 1.0000x reference)
"""Column-sum kernel for Trainium2: out[d] = sum_r x[r, d].

x is [8192, 4096] f32. Rows are sharded across 8 NeuronCores (1024 rows
each). Each core loads its shard as 8 contiguous [128, 4096] tiles,
reduces across the 128 partitions with a ones-vector matmul on the
tensor engine (accumulating over the 8 row-tiles in PSUM), and writes a
[1, 4096] partial. The host sums the 8 partials.
"""

import numpy as np

M_CORES = 8
ROWS, D = 8192, 4096
ROWS_PER_CORE = ROWS // M_CORES  # 1024
P = 128
K_TILES = ROWS_PER_CORE // P  # 8
NCHUNK = 512  # fp32 PSUM bank capacity / max fp32 moving free dim
N_TILES = D // NCHUNK  # 8

_nc_cache = None


def _build():
    import concourse.tile as tile
    from concourse import bacc, mybir

    nc = bacc.Bacc(None)
    x = nc.declare_dram_parameter(
        "x", [ROWS_PER_CORE, D], mybir.dt.float32, isOutput=False
    )
    out = nc.declare_dram_parameter("out", [1, D], mybir.dt.float32, isOutput=True)

    with tile.TileContext(nc) as tc:
        with (
            tc.tile_pool(name="xpool", bufs=8) as xpool,
            tc.tile_pool(name="singles", bufs=1) as singles,
            tc.tile_pool(name="psum", bufs=1, space="PSUM") as psum_pool,
        ):
            ones = singles.tile([P, 1], mybir.dt.float32)
            nc.any.memset(ones[:], 1.0)

            psums = [
                psum_pool.tile([1, NCHUNK], mybir.dt.float32, name=f"ps{n}", tag=f"ps{n}")
                for n in range(N_TILES)
            ]

            for k in range(K_TILES):
                xt = xpool.tile([P, D], mybir.dt.float32)
                nc.sync.dma_start(xt[:], x[k * P : (k + 1) * P, :])
                for n in range(N_TILES):
                    nc.tensor.matmul(
                        psums[n][:1],
                        ones[:],
                        xt[:, n * NCHUNK : (n + 1) * NCHUNK],
                        start=(k == 0),
                        stop=(k == K_TILES - 1),
                    )

            osb = singles.tile([1, D], mybir.dt.float32)
            for n in range(N_TILES):
                nc.vector.tensor_copy(osb[:, n * NCHUNK : (n + 1) * NCHUNK], psums[n][:1])
            nc.sync.dma_start(out[:, :], osb[:])

    nc.compile()
    return nc


def _get_nc():
    global _nc_cache
    if _nc_cache is None:
        _nc_cache = _build()
    return _nc_cache


def _run(x_np: np.ndarray, **run_kwargs):
    from concourse.bass_utils import run_bass_kernel_spmd

    nc = _get_nc()
    shards = np.split(x_np, M_CORES, axis=0)
    in_maps = [{"x": np.ascontiguousarray(s)} for s in shards]
    return run_bass_kernel_spmd(nc, in_maps, list(range(M_CORES)), **run_kwargs)


def kernel(x) -> np.ndarray:
    x_np = np.ascontiguousarray(np.asarray(x), dtype=np.float32)
    assert x_np.shape == (ROWS, D), x_np.shape
    res = _run(x_np)
    partials = np.stack([r["out"][0] for r in res.results])
    return partials.sum(axis=0, dtype=np.float32)


# revision 11
# speedup vs baseline: 1.0351x; 1.0351x over previous
"""Column-sum kernel for Trainium2: out[d] = sum_r x[r, d].

x is [8192, 4096] f32. Rows are sharded across 8 NeuronCores (1024 rows
each). Each core loads its shard as 8 contiguous [128, 4096] tiles,
folds them into one [128, 4096] accumulator with elementwise adds on
the vector engine (hidden under the DMA stream — fp32 PE matmul runs
at half rate, so streaming everything through the PE is 8x more PE
work), then does the final 128-partition reduce with a ones-vector
matmul (8 chunks of 512 into separate PSUM banks), copies PSUM out and
writes a [1, 4096] partial. The host sums the 8 partials.
"""

import numpy as np

M_CORES = 8
ROWS, D = 8192, 4096
ROWS_PER_CORE = ROWS // M_CORES  # 1024
P = 128
K_TILES = ROWS_PER_CORE // P  # 8
NCHUNK = 512  # fp32 PSUM bank capacity / max fp32 moving free dim
N_TILES = D // NCHUNK  # 8

_nc_cache = None


def _build():
    import concourse.tile as tile
    from concourse import bacc, mybir

    nc = bacc.Bacc(None)
    x = nc.declare_dram_parameter(
        "x", [ROWS_PER_CORE, D], mybir.dt.float32, isOutput=False
    )
    out = nc.declare_dram_parameter("out", [1, D], mybir.dt.float32, isOutput=True)

    with tile.TileContext(nc) as tc:
        with (
            tc.tile_pool(name="xpool", bufs=6) as xpool,
            tc.tile_pool(name="accpool", bufs=2) as accpool,
            tc.tile_pool(name="singles", bufs=1) as singles,
            tc.tile_pool(name="psum", bufs=1, space="PSUM") as psum_pool,
        ):
            ones = singles.tile([P, 1], mybir.dt.float32)
            nc.any.memset(ones[:], 1.0)

            psums = [
                psum_pool.tile(
                    [1, NCHUNK], mybir.dt.float32, name=f"ps{n}", tag=f"ps{n}"
                )
                for n in range(N_TILES)
            ]

            xts = []
            for k in range(K_TILES):
                xt = xpool.tile([P, D], mybir.dt.float32, name=f"xt{k}", tag="xt")
                nc.sync.dma_start(xt[:], x[k * P : (k + 1) * P, :])
                xts.append(xt)

            # Running sum of the 8 row-tiles on DVE; each add completes
            # well within the ~5us DMA inter-arrival, so only the last
            # add is on the critical path.
            acc = accpool.tile([P, D], mybir.dt.float32, name="acc0", tag="acc")
            nc.vector.tensor_add(acc[:], xts[0][:], xts[1][:])
            for k in range(2, K_TILES):
                nxt = accpool.tile([P, D], mybir.dt.float32, name=f"acc{k}", tag="acc")
                nc.vector.tensor_add(nxt[:], acc[:], xts[k][:])
                acc = nxt

            # Partition reduce: one complete matmul group per PSUM bank so
            # the PSUM->SBUF copies pipeline behind the matmuls.
            osb = singles.tile([1, D], mybir.dt.float32)
            for n in range(N_TILES):
                nc.tensor.matmul(
                    psums[n][:1],
                    ones[:],
                    acc[:, n * NCHUNK : (n + 1) * NCHUNK],
                    start=True,
                    stop=True,
                )
                nc.vector.tensor_copy(osb[:, n * NCHUNK : (n + 1) * NCHUNK], psums[n][:1])

            nc.sync.dma_start(out[:, :], osb[:])

    nc.compile()
    return nc


def _get_nc():
    global _nc_cache
    if _nc_cache is None:
        _nc_cache = _build()
    return _nc_cache


def _run(x_np: np.ndarray, **run_kwargs):
    from concourse.bass_utils import run_bass_kernel_spmd

    nc = _get_nc()
    shards = np.split(x_np, M_CORES, axis=0)
    in_maps = [{"x": np.ascontiguousarray(s)} for s in shards]
    return run_bass_kernel_spmd(nc, in_maps, list(range(M_CORES)), **run_kwargs)


def kernel(x) -> np.ndarray:
    x_np = np.ascontiguousarray(np.asarray(x), dtype=np.float32)
    assert x_np.shape == (ROWS, D), x_np.shape
    res = _run(x_np)
    partials = np.stack([r["out"][0] for r in res.results])
    return partials.sum(axis=0, dtype=np.float32)


# revision 12
# speedup vs baseline: 1.1900x; 1.1496x over previous
"""Column-sum kernel for Trainium2: out[d] = sum_r x[r, d].

x is [8192, 4096] f32. Rows are sharded across 8 NeuronCores (1024 rows
each). Per core, the shard is loaded as 8 contiguous [128, 4096] tiles.
fp32 PE matmul runs at half rate (LOW_HIGH double pass), so streaming
all 8 tiles through the PE is PE-bound; folding everything on DVE
leaves the PE cold and serializes a long tail. Instead the work is
split: even tiles go straight into the PE ones-matmul accumulation
groups (paced by DMA arrivals, which keeps the PE HAM-warm), odd tiles
are folded on DVE. The final DVE add is emitted per 512-column chunk so
the group-closing matmuls and PSUM copies pipeline right behind it.
Host sums the 8 per-core [1, 4096] partials.
"""

import numpy as np

M_CORES = 8
ROWS, D = 8192, 4096
ROWS_PER_CORE = ROWS // M_CORES  # 1024
P = 128
K_TILES = ROWS_PER_CORE // P  # 8
NCHUNK = 512  # fp32 PSUM bank capacity / max fp32 moving free dim
N_TILES = D // NCHUNK  # 8

_nc_cache = None


def _build():
    import concourse.tile as tile
    from concourse import bacc, mybir

    nc = bacc.Bacc(None)
    x = nc.declare_dram_parameter(
        "x", [ROWS_PER_CORE, D], mybir.dt.float32, isOutput=False
    )
    out = nc.declare_dram_parameter("out", [1, D], mybir.dt.float32, isOutput=True)

    with tile.TileContext(nc) as tc:
        with (
            tc.tile_pool(name="xpool", bufs=8) as xpool,
            tc.tile_pool(name="accpool", bufs=2) as accpool,
            tc.tile_pool(name="singles", bufs=1) as singles,
            tc.tile_pool(name="psum", bufs=1, space="PSUM") as psum_pool,
        ):
            ones = singles.tile([P, 1], mybir.dt.float32)
            nc.vector.memset(ones[:], 1.0)

            psums = [
                psum_pool.tile(
                    [1, NCHUNK], mybir.dt.float32, name=f"ps{n}", tag=f"ps{n}"
                )
                for n in range(N_TILES)
            ]

            xts = []
            for k in range(K_TILES):
                xt = xpool.tile([P, D], mybir.dt.float32, name=f"xt{k}", tag="xt")
                nc.sync.dma_start(xt[:], x[k * P : (k + 1) * P, :])
                xts.append(xt)

            def chunk(ap, n):
                return ap[:, n * NCHUNK : (n + 1) * NCHUNK]

            # Even tiles stream straight through the PE as they arrive.
            for ki, k in enumerate((0, 2, 4, 6)):
                for n in range(N_TILES):
                    nc.tensor.matmul(
                        psums[n][:1],
                        ones[:],
                        chunk(xts[k], n),
                        start=(ki == 0),
                        stop=False,
                    )

            # Odd tiles fold on DVE; each add hides under the ~5us DMA
            # inter-arrival so only the last one is on the critical path.
            a13 = accpool.tile([P, D], mybir.dt.float32, name="a13", tag="acc")
            nc.vector.tensor_add(a13[:], xts[1][:], xts[3][:])
            a135 = accpool.tile([P, D], mybir.dt.float32, name="a135", tag="acc")
            nc.vector.tensor_add(a135[:], a13[:], xts[5][:])

            # Last add chunked so matmul/copy pipeline behind it.
            afin = accpool.tile([P, D], mybir.dt.float32, name="afin", tag="acc")
            osb = singles.tile([1, D], mybir.dt.float32)
            for n in range(N_TILES):
                nc.vector.tensor_add(chunk(afin, n), chunk(a135, n), chunk(xts[7], n))
                nc.tensor.matmul(
                    psums[n][:1], ones[:], chunk(afin, n), start=False, stop=True
                )
                nc.vector.tensor_copy(chunk(osb, n), psums[n][:1])

            nc.sync.dma_start(out[:, :], osb[:])

    nc.compile()
    return nc


def _get_nc():
    global _nc_cache
    if _nc_cache is None:
        _nc_cache = _build()
    return _nc_cache


def _run(x_np: np.ndarray, **run_kwargs):
    from concourse.bass_utils import run_bass_kernel_spmd

    nc = _get_nc()
    shards = np.split(x_np, M_CORES, axis=0)
    in_maps = [{"x": np.ascontiguousarray(s)} for s in shards]
    return run_bass_kernel_spmd(nc, in_maps, list(range(M_CORES)), **run_kwargs)


def kernel(x) -> np.ndarray:
    x_np = np.ascontiguousarray(np.asarray(x), dtype=np.float32)
    assert x_np.shape == (ROWS, D), x_np.shape
    res = _run(x_np)
    partials = np.stack([r["out"][0] for r in res.results])
    return partials.sum(axis=0, dtype=np.float32)
